# revision 7
# baseline (speedup 1.0000x reference)
"""Bass/Trainium2 kernel for BasicGNNLayer (COO SpMM + mean aggregation + residual).

    out = features + (segment_sum(features[col], row) / clip(deg, 1)) .

Strategy (8 NeuronCores, SPMD):
  - Destination-shard nodes: core m owns rows [m*12500, (m+1)*12500).
  - Host sorts edges by dst row, groups them into 128-node tiles, pads each
    tile's edge list to C chunks of 128 edges (C = global max, same for all
    cores so one static NEFF works for everyone).
  - Per tile the device does: one indirect-DMA gather of the 128*C source
    rows (bf16), builds a 0/1 selection matrix S[e,n] = (dst(e) == n) with a
    single vector-engine is_equal against an iota, and runs C chained
    matmuls S.T @ [G | 1] accumulating in PSUM -> per-node sums plus degree.
  - Epilogue: r = 1/max(deg,1); out = sum * r + features (one fused DVE op),
    all tiles written to one SBUF slab, single DMA out.
  - Full feature table is replicated to every core (bf16 gather table), so
    no collectives are needed.
"""

import math
import os
import sys

for _p in ("/opt/trn_rl_repo", "/root/.axon_site/_ro/trn_rl_repo"):
    if os.path.isdir(_p) and _p not in sys.path:
        sys.path.insert(0, _p)

import numpy as np
import ml_dtypes

P = 128  # SBUF partitions


# ---------------------------------------------------------------- host side


def preprocess(features, row, col, n_cores):
    """Build per-core input maps. Returns (in_maps, meta)."""
    N, D = features.shape
    E = row.shape[0]
    # nodes per core, rounded up to a multiple of P so tiles never straddle
    # core boundaries (the last core simply owns fewer real rows)
    npc = ((N + n_cores - 1) // n_cores + P - 1) // P * P
    T = npc // P  # 128-node tiles per core
    npc_pad = T * P

    row = np.asarray(row).astype(np.int64)
    col = np.asarray(col).astype(np.int64)

    order = np.argsort(row, kind="stable")
    rs = row[order]
    cs = col[order]

    # per-core contiguous ranges in the sorted edge list
    bounds = np.searchsorted(rs, np.arange(n_cores + 1) * npc)

    # global tile id for every edge and chunk count C
    gtile = rs // P  # global 128-node tile (same as per-core tile since npc % P == 0 not required; see below)
    # NOTE: tiles must not straddle cores. Guaranteed iff npc % P == 0.
    assert npc % P == 0, "nodes-per-core must be a multiple of 128"
    tile_counts = np.bincount(gtile, minlength=n_cores * T)
    C = max(1, int((tile_counts.max() + P - 1) // P))

    feats_bf = np.ascontiguousarray(features.astype(ml_dtypes.bfloat16))
    iota = np.ascontiguousarray(
        np.broadcast_to(np.arange(P, dtype=np.float32), (P, P))
    ).astype(ml_dtypes.bfloat16)

    in_maps = []
    for m in range(n_cores):
        lo, hi = bounds[m], bounds[m + 1]
        local = rs[lo:hi] - m * npc  # in [0, npc)
        ccol = cs[lo:hi]
        tile = local // P
        # position of each edge within its tile (edges are tile-sorted)
        tstart = np.zeros(T, np.int64)
        cnt = np.bincount(tile, minlength=T)
        tstart[1:] = np.cumsum(cnt)[:-1]
        pos = np.arange(local.shape[0]) - tstart[tile]
        chunk = pos // P
        pp = pos % P
        slot = tile * C + chunk  # column in [P, T*C] layout

        colT = np.zeros((P, T * C), np.int32)
        rowv = np.full((P, T * C), -1.0, np.float32)
        colT[pp, slot] = ccol
        rowv[pp, slot] = (local - tile * P).astype(np.float32)

        base = m * npc
        valid = max(0, min(npc, N - base))
        slab = np.zeros((npc_pad, D), np.float32)
        slab[:valid] = features[base : base + valid]
        feats_loc = np.ascontiguousarray(
            slab.reshape(T, P, D).transpose(1, 0, 2).reshape(P, T * D)
        )

        in_maps.append(
            {
                "feats_bf": feats_bf,
                "feats_loc": feats_loc,
                "colT": colT,
                "rowv": rowv.astype(ml_dtypes.bfloat16),
                "iota": iota,
            }
        )

    meta = dict(N=N, D=D, E=E, npc=npc, T=T, C=C, n_cores=n_cores)
    return in_maps, meta


def postprocess(results, meta):
    """Stitch per-core [P, T*D] outputs back to [N, D]."""
    N, D, npc, T = meta["N"], meta["D"], meta["npc"], meta["T"]
    outs = []
    for m, res in enumerate(results):
        o = res["out"].reshape(P, T, D).transpose(1, 0, 2).reshape(T * P, D)
        valid = max(0, min(npc, N - m * npc))
        outs.append(o[:valid])
    return np.concatenate(outs, axis=0)


# -------------------------------------------------------------- device side


def build(meta):
    import concourse.bass as bass
    import concourse.bacc as bacc
    import concourse.mybir as mybir
    from concourse.tile import TileContext

    N, D, T, C = meta["N"], meta["D"], meta["T"], meta["C"]
    bf16 = mybir.dt.bfloat16
    f32 = mybir.dt.float32

    nc = bacc.Bacc()
    fb = nc.dram_tensor("feats_bf", [N, D], bf16, kind="ExternalInput")
    fl = nc.dram_tensor("feats_loc", [P, T * D], f32, kind="ExternalInput")
    ct = nc.dram_tensor("colT", [P, T * C], mybir.dt.int32, kind="ExternalInput")
    rv = nc.dram_tensor("rowv", [P, T * C], bf16, kind="ExternalInput")
    io = nc.dram_tensor("iota", [P, P], bf16, kind="ExternalInput")
    ot = nc.dram_tensor("out", [P, T * D], f32, kind="ExternalOutput")

    with TileContext(nc) as tc:
        with (
            tc.tile_pool(name="const", bufs=1) as cpool,
            tc.tile_pool(name="gat", bufs=3) as gpool,
            tc.tile_pool(name="sel", bufs=3) as spool,
            tc.tile_pool(name="eplg", bufs=4) as epool,
            tc.tile_pool(name="acc", bufs=4, space="PSUM") as ppool,
        ):
            iota_sb = cpool.tile([P, P], bf16, tag="iota")
            nc.sync.dma_start(out=iota_sb[:, :], in_=io[:, :])
            col_sb = cpool.tile([P, T * C], mybir.dt.int32, tag="colsb")
            nc.sync.dma_start(out=col_sb[:, :], in_=ct[:, :])
            row_sb = cpool.tile([P, T * C], bf16, tag="rowsb")
            nc.sync.dma_start(out=row_sb[:, :], in_=rv[:, :])
            slab_sb = cpool.tile([P, T * D], f32, tag="slab")
            nc.sync.dma_start(out=slab_sb[:, :], in_=fl[:, :])
            out_sb = cpool.tile([P, T * D], f32, tag="outsb")

            for t in range(T):
                # gather G[p, c, 0:D] = feats_bf[colT[p, t*C+c], :]
                G = gpool.tile([P, C, D + 2], bf16, tag="G")
                nc.vector.memset(G[:, :, D : D + 2], 1.0)
                for c in range(C):
                    nc.gpsimd.indirect_dma_start(
                        out=G[:, c, 0:D],
                        out_offset=None,
                        in_=fb[:, :],
                        in_offset=bass.IndirectOffsetOnAxis(
                            ap=col_sb[:, t * C + c : t * C + c + 1], axis=0
                        ),
                    )
                # S[p, c, j] = (rowv[p, t*C+c] == j)
                S = spool.tile([P, C, P], bf16, tag="S")
                nc.vector.tensor_tensor(
                    out=S[:, :, :],
                    in0=row_sb[:, t * C : (t + 1) * C]
                    .unsqueeze(-1)
                    .to_broadcast([P, C, P]),
                    in1=iota_sb[:, :].unsqueeze(1).to_broadcast([P, C, P]),
                    op=mybir.AluOpType.is_equal,
                )
                # psum[n, 0:D] = sum_e S[e,n] G[e,:]; psum[n, D] = deg(n)
                psum = ppool.tile([P, D + 1], f32, tag="psum")
                for c in range(C):
                    nc.tensor.matmul(
                        out=psum[:, :],
                        lhsT=S[:, c, :],
                        rhs=G[:, c, 0 : D + 1],
                        start=(c == 0),
                        stop=(c == C - 1),
                    )
                # r = 1 / max(deg, 1);  out = psum * r + feats_loc
                r = epool.tile([P, 1], f32, tag="recip")
                nc.vector.tensor_scalar_max(out=r[:, :], in0=psum[:, D : D + 1], scalar1=1.0)
                nc.vector.reciprocal(out=r[:, :], in_=r[:, :])
                nc.vector.scalar_tensor_tensor(
                    out=out_sb[:, t * D : (t + 1) * D],
                    in0=psum[:, 0:D],
                    scalar=r[:, 0:1],
                    in1=slab_sb[:, t * D : (t + 1) * D],
                    op0=mybir.AluOpType.mult,
                    op1=mybir.AluOpType.add,
                )
            nc.sync.dma_start(out=ot[:, :], in_=out_sb[:, :])
    nc.finalize()
    return nc


# ----------------------------------------------------------------- entry


def kernel(features, row, col):
    features = np.asarray(features, dtype=np.float32)
    n_cores = 8
    in_maps, meta = preprocess(features, row, col, n_cores)
    nc = build(meta)

    from concourse.bass_utils import run_bass_kernel_spmd

    res = run_bass_kernel_spmd(nc, in_maps, core_ids=list(range(n_cores)))
    return postprocess(res.results, meta)


if __name__ == "__main__":
    # smoke test with small random data
    rng = np.random.default_rng(0)
    N, D, E = 2048, 64, 16384
    feats = rng.standard_normal((N, D), dtype=np.float32)
    row = rng.integers(0, N, E, dtype=np.int32)
    col = rng.integers(0, N, E, dtype=np.int32)
    out = kernel(feats, row, col)
    print("out", out.shape, out.dtype)


# revision 12
# speedup vs baseline: 1.2809x; 1.2809x over previous
"""Bass/Trainium2 kernel for BasicGNNLayer (COO SpMM + mean aggregation + residual).

    out = features + (segment_sum(features[col], row) / clip(deg, 1)) .

Strategy (8 NeuronCores, SPMD, no collectives):
  - Destination-shard nodes: core m owns a 12544-row slab (98 tiles of 128).
  - Host sorts edges by dst row into 128-node tiles; within a tile edges are
    bucketed by source shard (4 shards of <=25088 rows for int16 dma_gather
    indices) and padded to Cs chunks of 128 edges per shard.
  - Gather table: [N, 128] bf16, cols 0:64 = features, col 64 = 1.0 (so the
    segmented-sum matmul also produces the degree), rest zero.
  - Per 7-tile group x shard: ONE dma_gather (g*Cs*128 indices, 256B/row)
    into SBUF. Per tile: one vector is_equal builds S[e,n] = (dst(e)==n),
    then 4*Cs chained matmuls S.T @ G[:, :, 0:65] accumulate sums+deg in
    PSUM. Epilogue fuses 1/max(deg,1) scaling + residual add in one DVE op.
  - Full table is replicated to every core; per-core index/rowv arrays differ.
"""

import math
import os
import sys

for _p in ("/opt/trn_rl_repo", "/root/.axon_site/_ro/trn_rl_repo"):
    if os.path.isdir(_p) and _p not in sys.path:
        sys.path.insert(0, _p)

import numpy as np
import ml_dtypes

P = 128  # SBUF partitions
NSHARD = 4  # gather-table shards (int16 index limit)


def _pick_grp(T):
    for g in range(8, 0, -1):
        if T % g == 0:
            return g
    return 1


def _shard_size(N):
    s = (N + NSHARD - 1) // NSHARD
    assert s <= 32768, "int16 index limit"
    return s


# ---------------------------------------------------------------- host side


def preprocess(features, row, col, n_cores):
    """Build per-core input maps. Returns (in_maps, meta)."""
    N, D = features.shape
    E = row.shape[0]
    npc = ((N + n_cores - 1) // n_cores + P - 1) // P * P
    T = npc // P
    GRP = _pick_grp(T)
    NG = T // GRP  # gather groups per core
    SS = _shard_size(N)

    row = np.asarray(row).astype(np.int64)
    col = np.asarray(col).astype(np.int64)

    # sort edges by (dst tile, src shard) bucket id -> contiguous buckets
    shard = col // SS
    gts0 = (row // P) * NSHARD + shard  # global (tile, shard) id
    order = np.argsort(gts0, kind="stable")
    rs = row[order]
    cs = col[order]
    sh = shard[order]
    gts = gts0[order]

    # per-(tile, shard) counts -> global max chunk count Cs
    n_gts = (n_cores * T) * NSHARD
    cnt = np.bincount(gts, minlength=n_gts)
    Cs = max(1, int((cnt.max() + P - 1) // P))
    CT = NSHARD * Cs  # chunks per tile

    # gather table: [N, 128] bf16, features | 1.0 | zeros
    tab = np.zeros((N, 2 * D), ml_dtypes.bfloat16)
    tab[:, :D] = features.astype(ml_dtypes.bfloat16)
    tab[:, D] = 1.0
    iota = np.ascontiguousarray(
        np.broadcast_to(np.arange(P, dtype=np.float32), (P, P))
    ).astype(ml_dtypes.bfloat16)

    # slot assignment (global, vectorized): position of each edge within its
    # (tile, shard) bucket
    starts = np.zeros(n_gts, np.int64)
    starts[1:] = np.cumsum(cnt)[:-1]
    pos = np.arange(E) - starts[gts]
    chunk = pos // P  # chunk within (tile, shard)
    pp = pos % P

    # core boundaries from the (sorted) bucket ids: core m owns buckets
    # [m*T*NSHARD, (m+1)*T*NSHARD)
    bounds = np.searchsorted(gts, np.arange(n_cores + 1) * T * NSHARD)

    in_maps = []
    for m in range(n_cores):
        lo, hi = bounds[m], bounds[m + 1]
        local = rs[lo:hi] - m * npc
        tile = local // P
        shd = sh[lo:hi]
        chk = chunk[lo:hi]
        ppp = pp[lo:hi]

        # rowv: [P, T*CT] bf16, chunk column index = t*CT + s*Cs + c
        ccol = tile * CT + shd * Cs + chk
        rowv = np.full((P, T * CT), -1.0, np.float32)
        rowv[ppp, ccol] = (local - tile * P).astype(np.float32)

        # int16 gather indices, flat order: call (g, s) covers
        # [t in g*GRP..(g+1)*GRP) x c in 0..Cs) x p], flat q = (t_in_g*Cs+c)*128+p
        idx_flat = np.zeros(T * CT * P, np.int16)  # pad = 0
        g = tile // GRP
        tin = tile % GRP
        call = g * NSHARD + shd
        q = (call * GRP * Cs + tin * Cs + chk) * P + ppp
        idx_flat[q] = (cs[lo:hi] - shd * SS).astype(np.int16)  # noqa
        # wrap: per call, flat i -> [i % 16, i // 16], then replicate 8x to 128
        CL = GRP * Cs * P  # idxs per call
        ncalls = NG * NSHARD
        w = idx_flat.reshape(ncalls, CL // 16, 16)
        w = np.ascontiguousarray(np.transpose(w, (2, 0, 1))).reshape(16, ncalls * (CL // 16))
        idx16 = np.ascontiguousarray(np.tile(w, (8, 1)))  # [128, T*CT*P/16]

        base = m * npc
        valid = max(0, min(npc, N - base))
        slab = np.zeros((T * P, D), np.float32)
        slab[:valid] = features[base : base + valid]
        feats_loc = np.ascontiguousarray(
            slab.reshape(T, P, D).transpose(1, 0, 2).reshape(P, T * D)
        )

        in_maps.append(
            {
                "tab": tab,
                "feats_loc": feats_loc,
                "idx16": idx16,
                "rowv": rowv.astype(ml_dtypes.bfloat16),
                "iota": iota,
            }
        )

    meta = dict(N=N, D=D, E=E, npc=npc, T=T, Cs=Cs, CT=CT, NG=NG, SS=SS,
                GRP=GRP, n_cores=n_cores)
    return in_maps, meta


def postprocess(results, meta):
    """Stitch per-core [P, T*D] outputs back to [N, D]."""
    N, D, npc, T = meta["N"], meta["D"], meta["npc"], meta["T"]
    outs = []
    for m, res in enumerate(results):
        o = res["out"].reshape(P, T, D).transpose(1, 0, 2).reshape(T * P, D)
        valid = max(0, min(npc, N - m * npc))
        outs.append(o[:valid])
    return np.concatenate(outs, axis=0)


# -------------------------------------------------------------- device side


def build(meta):
    import concourse.bass as bass
    import concourse.bacc as bacc
    import concourse.mybir as mybir
    from concourse.tile import TileContext

    N, D, T = meta["N"], meta["D"], meta["T"]
    Cs, CT, NG, SS = meta["Cs"], meta["CT"], meta["NG"], meta["SS"]
    GRP = meta["GRP"]
    W = 2 * D  # table row width (128)
    bf16 = mybir.dt.bfloat16
    f32 = mybir.dt.float32

    nc = bacc.Bacc()
    tab = nc.dram_tensor("tab", [N, W], bf16, kind="ExternalInput")
    fl = nc.dram_tensor("feats_loc", [P, T * D], f32, kind="ExternalInput")
    ix = nc.dram_tensor("idx16", [P, T * CT * P // 16], mybir.dt.int16, kind="ExternalInput")
    rv = nc.dram_tensor("rowv", [P, T * CT], bf16, kind="ExternalInput")
    io = nc.dram_tensor("iota", [P, P], bf16, kind="ExternalInput")
    ot = nc.dram_tensor("out", [P, T * D], f32, kind="ExternalOutput")

    CL = GRP * Cs * P  # indices per gather call
    CLW = CL // 16  # idx16 cols per call

    with TileContext(nc) as tc:
        with (
            tc.tile_pool(name="const", bufs=1) as cpool,
            tc.tile_pool(name="gat", bufs=2) as gpool,
            tc.tile_pool(name="sel", bufs=3) as spool,
            tc.tile_pool(name="eplg", bufs=4) as epool,
            tc.tile_pool(name="acc", bufs=4, space="PSUM") as ppool,
        ):
            iota_sb = cpool.tile([P, P], bf16, tag="iota")
            nc.sync.dma_start(out=iota_sb[:, :], in_=io[:, :])
            idx_sb = cpool.tile([P, T * CT * P // 16], mybir.dt.int16, tag="idxsb")
            nc.sync.dma_start(out=idx_sb[:, :], in_=ix[:, :])
            row_sb = cpool.tile([P, T * CT], bf16, tag="rowsb")
            nc.sync.dma_start(out=row_sb[:, :], in_=rv[:, :])
            slab_sb = cpool.tile([P, T * D], f32, tag="slab")
            nc.sync.dma_start(out=slab_sb[:, :], in_=fl[:, :])
            out_sb = cpool.tile([P, T * D], f32, tag="outsb")

            for g in range(NG):
                # one gather per shard for this 7-tile group
                Gs = []
                for s in range(NSHARD):
                    Gt = gpool.tile([P, GRP * Cs, W], bf16, tag=f"G{s}")
                    call = g * NSHARD + s
                    nc.gpsimd.dma_gather(
                        out_ap=Gt[:, :, :],
                        in_ap=tab[s * SS : min(N, (s + 1) * SS), :],
                        idxs_ap=idx_sb[:, call * CLW : (call + 1) * CLW],
                        num_idxs=CL,
                        num_idxs_reg=CL,
                        elem_size=W,
                        single_packet=False,
                    )
                    Gs.append(Gt)
                for tin in range(GRP):
                    t = g * GRP + tin
                    # S[p, k, j] = (rowv[p, t*CT + k] == j)
                    S = spool.tile([P, CT, P], bf16, tag="S")
                    nc.vector.tensor_tensor(
                        out=S[:, :, :],
                        in0=row_sb[:, t * CT : (t + 1) * CT]
                        .unsqueeze(-1)
                        .to_broadcast([P, CT, P]),
                        in1=iota_sb[:, :].unsqueeze(1).to_broadcast([P, CT, P]),
                        op=mybir.AluOpType.is_equal,
                    )
                    psum = ppool.tile([P, D + 1], f32, tag="psum")
                    k = 0
                    for s in range(NSHARD):
                        for c in range(Cs):
                            nc.tensor.matmul(
                                out=psum[:, :],
                                lhsT=S[:, s * Cs + c, :],
                                rhs=Gs[s][:, tin * Cs + c, 0 : D + 1],
                                start=(k == 0),
                                stop=(k == CT - 1),
                            )
                            k += 1
                    r = epool.tile([P, 1], f32, tag="recip")
                    nc.vector.tensor_scalar_max(
                        out=r[:, :], in0=psum[:, D : D + 1], scalar1=1.0
                    )
                    nc.vector.reciprocal(out=r[:, :], in_=r[:, :])
                    nc.vector.scalar_tensor_tensor(
                        out=out_sb[:, t * D : (t + 1) * D],
                        in0=psum[:, 0:D],
                        scalar=r[:, 0:1],
                        in1=slab_sb[:, t * D : (t + 1) * D],
                        op0=mybir.AluOpType.mult,
                        op1=mybir.AluOpType.add,
                    )
            nc.sync.dma_start(out=ot[:, :], in_=out_sb[:, :])
    nc.finalize()
    return nc


# ----------------------------------------------------------------- entry


def kernel(features, row, col):
    features = np.asarray(features, dtype=np.float32)
    n_cores = 8
    in_maps, meta = preprocess(features, row, col, n_cores)
    nc = build(meta)

    from concourse.bass_utils import run_bass_kernel_spmd

    res = run_bass_kernel_spmd(nc, in_maps, core_ids=list(range(n_cores)))
    return postprocess(res.results, meta)


if __name__ == "__main__":
    rng = np.random.default_rng(0)
    N, D, E = 7168, 64, 57344
    feats = rng.standard_normal((N, D), dtype=np.float32)
    row = rng.integers(0, N, E, dtype=np.int32)
    col = rng.integers(0, N, E, dtype=np.int32)
    out = kernel(feats, row, col)

    n = N
    gathered = feats[col]
    summed = np.zeros((n, D), np.float32)
    np.add.at(summed, row, gathered)
    deg = np.clip(np.bincount(row, minlength=n).astype(np.float32), 1.0, None)
    exp = feats + summed / deg[:, None]
    rel = np.linalg.norm(out - exp) / np.linalg.norm(exp)
    print("rel err:", rel, "PASS" if rel < 5e-3 else "FAIL")


# revision 14
# speedup vs baseline: 1.3636x; 1.0646x over previous
"""Bass/Trainium2 kernel for BasicGNNLayer (COO SpMM + mean aggregation + residual).

    out = features + (segment_sum(features[col], row) / clip(deg, 1)) .

Strategy (8 NeuronCores, SPMD, no collectives):
  - Destination-shard nodes: core m owns a 12544-row slab (98 tiles of 128).
  - Host sorts edges by dst row into 128-node tiles; within a tile edges are
    bucketed by source shard (4 shards of <=25088 rows for int16 dma_gather
    indices) and padded to Cs chunks of 128 edges per shard.
  - Gather table: [N, 128] bf16, cols 0:64 = features, col 64 = 1.0 (so the
    segmented-sum matmul also produces the degree), rest zero.
  - Per 7-tile group x shard: ONE dma_gather (g*Cs*128 indices, 256B/row)
    into SBUF. Per tile: one vector is_equal builds S[e,n] = (dst(e)==n),
    then 4*Cs chained matmuls S.T @ G[:, :, 0:65] accumulate sums+deg in
    PSUM. Epilogue fuses 1/max(deg,1) scaling + residual add in one DVE op.
  - Full table is replicated to every core; per-core index/rowv arrays differ.
"""

import math
import os
import sys

for _p in ("/opt/trn_rl_repo", "/root/.axon_site/_ro/trn_rl_repo"):
    if os.path.isdir(_p) and _p not in sys.path:
        sys.path.insert(0, _p)

import numpy as np
import ml_dtypes

P = 128  # SBUF partitions
NSHARD = 4  # gather-table shards (int16 index limit)


def _pick_grp(T):
    for g in range(8, 0, -1):
        if T % g == 0:
            return g
    return 1


def _shard_size(N):
    s = (N + NSHARD - 1) // NSHARD
    assert s <= 32768, "int16 index limit"
    return s


# ---------------------------------------------------------------- host side


def preprocess(features, row, col, n_cores):
    """Build per-core input maps. Returns (in_maps, meta)."""
    N, D = features.shape
    E = row.shape[0]
    npc = ((N + n_cores - 1) // n_cores + P - 1) // P * P
    T = npc // P
    GRP = _pick_grp(T)
    NG = T // GRP  # gather groups per core
    SS = _shard_size(N)

    row = np.asarray(row).astype(np.int64)
    col = np.asarray(col).astype(np.int64)

    # sort edges by (dst tile, src shard) bucket id -> contiguous buckets
    shard = col // SS
    gts0 = (row // P) * NSHARD + shard  # global (tile, shard) id
    order = np.argsort(gts0, kind="stable")
    rs = row[order]
    cs = col[order]
    sh = shard[order]
    gts = gts0[order]

    # per-(tile, shard) counts -> global max chunk count Cs
    n_gts = (n_cores * T) * NSHARD
    cnt = np.bincount(gts, minlength=n_gts)
    Cs = max(1, int((cnt.max() + P - 1) // P))
    CT = NSHARD * Cs  # chunks per tile

    # gather table: [N, 128] bf16, features | 1.0 | zeros
    tab = np.zeros((N, 2 * D), ml_dtypes.bfloat16)
    tab[:, :D] = features.astype(ml_dtypes.bfloat16)
    tab[:, D] = 1.0
    iota = np.ascontiguousarray(
        np.broadcast_to(np.arange(P, dtype=np.float32), (P, P))
    ).astype(ml_dtypes.bfloat16)

    # slot assignment (global, vectorized): position of each edge within its
    # (tile, shard) bucket
    starts = np.zeros(n_gts, np.int64)
    starts[1:] = np.cumsum(cnt)[:-1]
    pos = np.arange(E) - starts[gts]
    chunk = pos // P  # chunk within (tile, shard)
    pp = pos % P

    # core boundaries from the (sorted) bucket ids: core m owns buckets
    # [m*T*NSHARD, (m+1)*T*NSHARD)
    bounds = np.searchsorted(gts, np.arange(n_cores + 1) * T * NSHARD)

    in_maps = []
    for m in range(n_cores):
        lo, hi = bounds[m], bounds[m + 1]
        local = rs[lo:hi] - m * npc
        tile = local // P
        shd = sh[lo:hi]
        chk = chunk[lo:hi]
        ppp = pp[lo:hi]

        # rowv: [P, T*CT] bf16, chunk column index = t*CT + s*Cs + c
        ccol = tile * CT + shd * Cs + chk
        rowv = np.full((P, T * CT), -1.0, np.float32)
        rowv[ppp, ccol] = (local - tile * P).astype(np.float32)

        # int16 gather indices, flat order: call (g, s) covers
        # [t in g*GRP..(g+1)*GRP) x c in 0..Cs) x p], flat q = (t_in_g*Cs+c)*128+p
        idx_flat = np.zeros(T * CT * P, np.int16)  # pad = 0
        g = tile // GRP
        tin = tile % GRP
        call = g * NSHARD + shd
        q = (call * GRP * Cs + tin * Cs + chk) * P + ppp
        idx_flat[q] = (cs[lo:hi] - shd * SS).astype(np.int16)  # noqa
        # wrap: per call, flat i -> [i % 16, i // 16], then replicate 8x to 128
        CL = GRP * Cs * P  # idxs per call
        ncalls = NG * NSHARD
        w = idx_flat.reshape(ncalls, CL // 16, 16)
        w = np.ascontiguousarray(np.transpose(w, (2, 0, 1))).reshape(16, ncalls * (CL // 16))
        idx16 = np.ascontiguousarray(np.tile(w, (8, 1)))  # [128, T*CT*P/16]

        base = m * npc
        valid = max(0, min(npc, N - base))
        slab = np.zeros((T * P, D), np.float32)
        slab[:valid] = features[base : base + valid]
        feats_loc = np.ascontiguousarray(
            slab.reshape(T, P, D).transpose(1, 0, 2).reshape(P, T * D)
        )

        in_maps.append(
            {
                "tab": tab,
                "feats_loc": feats_loc,
                "idx16": idx16,
                "rowv": rowv.astype(ml_dtypes.bfloat16),
                "iota": iota,
            }
        )

    meta = dict(N=N, D=D, E=E, npc=npc, T=T, Cs=Cs, CT=CT, NG=NG, SS=SS,
                GRP=GRP, n_cores=n_cores)
    return in_maps, meta


def postprocess(results, meta):
    """Stitch per-core [P, T*D] outputs back to [N, D]."""
    N, D, npc, T = meta["N"], meta["D"], meta["npc"], meta["T"]
    outs = []
    for m, res in enumerate(results):
        o = res["out"].reshape(P, T, D).transpose(1, 0, 2).reshape(T * P, D)
        valid = max(0, min(npc, N - m * npc))
        outs.append(o[:valid])
    return np.concatenate(outs, axis=0)


# -------------------------------------------------------------- device side


def build(meta):
    import concourse.bass as bass
    import concourse.bacc as bacc
    import concourse.mybir as mybir
    from concourse.tile import TileContext

    N, D, T = meta["N"], meta["D"], meta["T"]
    Cs, CT, NG, SS = meta["Cs"], meta["CT"], meta["NG"], meta["SS"]
    GRP = meta["GRP"]
    W = 2 * D  # table row width (128)
    bf16 = mybir.dt.bfloat16
    f32 = mybir.dt.float32

    nc = bacc.Bacc()
    tab = nc.dram_tensor("tab", [N, W], bf16, kind="ExternalInput")
    fl = nc.dram_tensor("feats_loc", [P, T * D], f32, kind="ExternalInput")
    ix = nc.dram_tensor("idx16", [P, T * CT * P // 16], mybir.dt.int16, kind="ExternalInput")
    rv = nc.dram_tensor("rowv", [P, T * CT], bf16, kind="ExternalInput")
    io = nc.dram_tensor("iota", [P, P], bf16, kind="ExternalInput")
    ot = nc.dram_tensor("out", [P, T * D], f32, kind="ExternalOutput")

    CL = GRP * Cs * P  # indices per gather call
    CLW = CL // 16  # idx16 cols per call

    with TileContext(nc) as tc:
        with (
            tc.tile_pool(name="const", bufs=1) as cpool,
            tc.tile_pool(name="gat", bufs=2) as gpool,
            tc.tile_pool(name="sel", bufs=3) as spool,
            tc.tile_pool(name="eplg", bufs=4) as epool,
            tc.tile_pool(name="acc", bufs=4, space="PSUM") as ppool,
        ):
            iota_sb = cpool.tile([P, P], bf16, tag="iota")
            nc.sync.dma_start(out=iota_sb[:, :], in_=io[:, :])
            idx_sb = cpool.tile([P, T * CT * P // 16], mybir.dt.int16, tag="idxsb")
            nc.sync.dma_start(out=idx_sb[:, :], in_=ix[:, :])
            row_sb = cpool.tile([P, T * CT], bf16, tag="rowsb")
            nc.sync.dma_start(out=row_sb[:, :], in_=rv[:, :])
            slab_sb = cpool.tile([P, T * D], f32, tag="slab")
            nc.sync.dma_start(out=slab_sb[:, :], in_=fl[:, :])
            out_sb = cpool.tile([P, T * D], f32, tag="outsb")

            for g in range(NG):
                # one gather per shard for this 7-tile group
                Gs = []
                for s in range(NSHARD):
                    Gt = gpool.tile([P, GRP * Cs, W], bf16, tag=f"G{s}")
                    call = g * NSHARD + s
                    nc.gpsimd.dma_gather(
                        out_ap=Gt[:, :, :],
                        in_ap=tab[s * SS : min(N, (s + 1) * SS), :],
                        idxs_ap=idx_sb[:, call * CLW : (call + 1) * CLW],
                        num_idxs=CL,
                        num_idxs_reg=CL,
                        elem_size=W,
                        single_packet=False,
                    )
                    Gs.append(Gt)
                for tin in range(GRP):
                    t = g * GRP + tin
                    # S[p, k, j] = (rowv[p, t*CT + k] == j)
                    S = spool.tile([P, CT, P], bf16, tag="S")
                    nc.vector.tensor_tensor(
                        out=S[:, :, :],
                        in0=row_sb[:, t * CT : (t + 1) * CT]
                        .unsqueeze(-1)
                        .to_broadcast([P, CT, P]),
                        in1=iota_sb[:, :].unsqueeze(1).to_broadcast([P, CT, P]),
                        op=mybir.AluOpType.is_equal,
                    )
                    psum = ppool.tile([P, D + 1], f32, tag="psum")
                    k = 0
                    for s in range(NSHARD):
                        for c in range(Cs):
                            nc.tensor.matmul(
                                out=psum[:, :],
                                lhsT=S[:, s * Cs + c, :],
                                rhs=Gs[s][:, tin * Cs + c, 0 : D + 1],
                                start=(k == 0),
                                stop=(k == CT - 1),
                            )
                            k += 1
                    r = epool.tile([P, 1], f32, tag="recip")
                    nc.vector.tensor_scalar_max(
                        out=r[:, :], in0=psum[:, D : D + 1], scalar1=1.0
                    )
                    nc.vector.reciprocal(out=r[:, :], in_=r[:, :])
                    nc.vector.scalar_tensor_tensor(
                        out=out_sb[:, t * D : (t + 1) * D],
                        in0=psum[:, 0:D],
                        scalar=r[:, 0:1],
                        in1=slab_sb[:, t * D : (t + 1) * D],
                        op0=mybir.AluOpType.mult,
                        op1=mybir.AluOpType.add,
                    )
            nc.sync.dma_start(out=ot[:, :], in_=out_sb[:, :])
    nc.finalize()
    return nc


# ----------------------------------------------------------------- entry


def kernel(features, row, col):
    features = np.asarray(features, dtype=np.float32)
    n_cores = 8
    in_maps, meta = preprocess(features, row, col, n_cores)
    nc = build(meta)

    from concourse.bass_utils import run_bass_kernel_spmd

    res = run_bass_kernel_spmd(nc, in_maps, core_ids=list(range(n_cores)))
    return postprocess(res.results, meta)


if __name__ == "__main__":
    rng = np.random.default_rng(0)
    N, D, E = 7168, 64, 57344
    feats = rng.standard_normal((N, D), dtype=np.float32)
    row = rng.integers(0, N, E, dtype=np.int32)
    col = rng.integers(0, N, E, dtype=np.int32)
    out = kernel(feats, row, col)

    n = N
    gathered = feats[col]
    summed = np.zeros((n, D), np.float32)
    np.add.at(summed, row, gathered)
    deg = np.clip(np.bincount(row, minlength=n).astype(np.float32), 1.0, None)
    exp = feats + summed / deg[:, None]
    rel = np.linalg.norm(out - exp) / np.linalg.norm(exp)
    print("rel err:", rel, "PASS" if rel < 5e-3 else "FAIL")


# revision 16
# speedup vs baseline: 1.4873x; 1.0907x over previous
"""Bass/Trainium2 kernel for BasicGNNLayer (COO SpMM + mean aggregation + residual).

    out = features + (segment_sum(features[col], row) / clip(deg, 1)) .

Strategy (8 NeuronCores, SPMD, no collectives):
  - Destination-shard nodes: core m owns a 12544-row slab (98 tiles of 128).
  - Host sorts edges by dst row into 128-node tiles; within a tile edges are
    bucketed by source shard (4 shards of <=25088 rows for int16 dma_gather
    indices). Each (tile, shard) bucket is capped at CAP chunks of 128 edges;
    the Poisson excess spills into per-(7-tile-group, shard) overflow chunks
    shared by all tiles of the group (their selection matrix zeroes foreign
    slots). This minimizes the index count the Q7 SWDGE must emit, which is
    the hard bottleneck (~8ns per index, data-independent).
  - Gather table: [N, 128] bf16, cols 0:64 = features, col 64 = 1.0 (so the
    segmented-sum matmul also produces the degree), rest zero.
  - Per (group, shard): ONE dma_gather of (GRP*CAP+GOC)*128 indices (256B
    rows). Per tile: one vector is_equal builds S[e,n] = (dst(e)==n) over its
    capped + overflow chunks, then chained matmuls S.T @ G[:, :, 0:65]
    accumulate sums+deg in PSUM. Epilogue fuses 1/max(deg,1) scaling +
    residual add in one DVE op.
"""

import os
import sys

for _p in ("/opt/trn_rl_repo", "/root/.axon_site/_ro/trn_rl_repo"):
    if os.path.isdir(_p) and _p not in sys.path:
        sys.path.insert(0, _p)

import numpy as np
import ml_dtypes

P = 128  # SBUF partitions
NSHARD = 4  # gather-table shards (int16 index limit)
CAP = 4  # capped chunks per (tile, shard) bucket


def _pick_grp(T):
    for g in range(8, 0, -1):
        if T % g == 0:
            return g
    return 1


def _shard_size(N):
    s = (N + NSHARD - 1) // NSHARD
    assert s <= 32768, "int16 index limit"
    return s


# ---------------------------------------------------------------- host side


def preprocess(features, row, col, n_cores):
    """Build per-core input maps. Returns (in_maps, meta)."""
    N, D = features.shape
    E = row.shape[0]
    npc = ((N + n_cores - 1) // n_cores + P - 1) // P * P
    T = npc // P
    GRP = _pick_grp(T)
    NG = T // GRP
    SS = _shard_size(N)

    row = np.asarray(row).astype(np.int64)
    col = np.asarray(col).astype(np.int64)

    shard = col // SS
    gts0 = (row // P) * NSHARD + shard  # global (tile, shard) bucket id
    order = np.argsort(gts0, kind="stable")
    rs = row[order]
    cs = col[order]
    sh = shard[order]
    gts = gts0[order]

    n_gts = (n_cores * T) * NSHARD
    cnt = np.bincount(gts, minlength=n_gts)
    Cs = max(1, int((cnt.max() + P - 1) // P))
    cap = min(CAP, Cs)

    # rank of each edge within its bucket
    starts = np.zeros(n_gts, np.int64)
    starts[1:] = np.cumsum(cnt)[:-1]
    pos = np.arange(E) - starts[gts]

    # overflow edges (pos >= cap*128) pool per (core, group, shard)
    ovf = pos >= cap * P
    gtile = rs // P  # global tile id
    core_of = gtile // T
    g_of = (gtile % T) // GRP
    pool = (core_of * NG + g_of) * NSHARD + sh  # global pool id
    n_pools = n_cores * NG * NSHARD
    ovf_pool = pool[ovf]
    po = np.argsort(ovf_pool, kind="stable")
    pcnt = np.bincount(ovf_pool, minlength=n_pools)
    pstart = np.zeros(n_pools, np.int64)
    pstart[1:] = np.cumsum(pcnt)[:-1]
    ovrank_sub = np.empty(ovf_pool.shape[0], np.int64)
    ovrank_sub[po] = np.arange(ovf_pool.shape[0]) - pstart[ovf_pool[po]]
    ovrank = np.zeros(E, np.int64)
    ovrank[np.where(ovf)[0]] = ovrank_sub
    GOC = int((pcnt.max() + P - 1) // P) if ovf.any() else 0

    CPT = NSHARD * cap  # capped chunks per tile
    CPG = NSHARD * GOC  # overflow chunks per group
    KT = CPT + CPG  # matmul chunks per tile (incl. group overflow)
    CC = GRP * cap + GOC  # gather chunks per (group, shard) call
    CL = CC * P  # indices per call
    CLW = CL // 16

    tab = np.zeros((N, 2 * D), ml_dtypes.bfloat16)
    tab[:, :D] = features.astype(ml_dtypes.bfloat16)
    tab[:, D] = 1.0
    iota = np.ascontiguousarray(
        np.broadcast_to(np.arange(P, dtype=np.float32), (P, P))
    ).astype(ml_dtypes.bfloat16)

    bounds = np.searchsorted(gts, np.arange(n_cores + 1) * T * NSHARD)

    in_maps = []
    for m in range(n_cores):
        lo, hi = bounds[m], bounds[m + 1]
        local = rs[lo:hi] - m * npc
        tile = local // P  # tile within core
        shd = sh[lo:hi]
        pp_ = pos[lo:hi]
        ov_ = ovf[lo:hi]
        ovr = ovrank[lo:hi]  # only valid where ov_
        g = tile // GRP
        tin = tile % GRP

        # --- capped slots
        cm = ~ov_
        cc = pp_[cm] // P
        cp = pp_[cm] % P
        # rowv column layout per tile t: [s*cap + c | CPT + s*GOC + co]
        rowv = np.full((P, T * KT), -1.0, np.float32)
        rowv[cp, (tile[cm] * KT + shd[cm] * cap + cc)] = (
            local[cm] - tile[cm] * P
        ).astype(np.float32)
        # idx flat position within call (g, s): (tin*cap + cc)*128 + p
        idx_flat = np.zeros(NG * NSHARD * CL, np.int16)
        call = g * NSHARD + shd
        q = (call[cm] * CC + tin[cm] * cap + cc) * P + cp
        idx_flat[q] = (cs[lo:hi][cm] - shd[cm] * SS).astype(np.int16)

        # --- overflow slots
        if GOC:
            co = ovr[ov_] // P
            op_ = ovr[ov_] % P
            rowv[op_, (tile[ov_] * KT + CPT + shd[ov_] * GOC + co)] = (
                local[ov_] - tile[ov_] * P
            ).astype(np.float32)
            qo = (call[ov_] * CC + GRP * cap + co) * P + op_
            idx_flat[qo] = (cs[lo:hi][ov_] - shd[ov_] * SS).astype(np.int16)

        ncalls = NG * NSHARD
        w = idx_flat.reshape(ncalls, CLW, 16)
        w = np.ascontiguousarray(np.transpose(w, (2, 0, 1))).reshape(16, ncalls * CLW)
        idx16 = np.ascontiguousarray(np.tile(w, (8, 1)))

        base = m * npc
        valid = max(0, min(npc, N - base))
        slab = np.zeros((T * P, D), np.float32)
        slab[:valid] = features[base : base + valid]
        feats_loc = np.ascontiguousarray(
            slab.reshape(T, P, D).transpose(1, 0, 2).reshape(P, T * D)
        )

        in_maps.append(
            {
                "tab": tab,
                "feats_loc": feats_loc,
                "idx16": idx16,
                "rowv": rowv.astype(ml_dtypes.bfloat16),
                "iota": iota,
            }
        )

    meta = dict(N=N, D=D, E=E, npc=npc, T=T, cap=cap, GOC=GOC, KT=KT, CC=CC,
                NG=NG, SS=SS, GRP=GRP, n_cores=n_cores)
    return in_maps, meta


def postprocess(results, meta):
    N, D, npc, T = meta["N"], meta["D"], meta["npc"], meta["T"]
    outs = []
    for m, res in enumerate(results):
        o = res["out"].reshape(P, T, D).transpose(1, 0, 2).reshape(T * P, D)
        valid = max(0, min(npc, N - m * npc))
        outs.append(o[:valid])
    return np.concatenate(outs, axis=0)


# -------------------------------------------------------------- device side


def build(meta):
    import concourse.bass as bass  # noqa: F401
    import concourse.bacc as bacc
    import concourse.mybir as mybir
    from concourse.tile import TileContext

    N, D, T = meta["N"], meta["D"], meta["T"]
    cap, GOC, KT, CC = meta["cap"], meta["GOC"], meta["KT"], meta["CC"]
    NG, SS, GRP = meta["NG"], meta["SS"], meta["GRP"]
    W = 2 * D  # table row width (128)
    bf16 = mybir.dt.bfloat16
    f32 = mybir.dt.float32

    nc = bacc.Bacc()
    tab = nc.dram_tensor("tab", [N, W], bf16, kind="ExternalInput")
    fl = nc.dram_tensor("feats_loc", [P, T * D], f32, kind="ExternalInput")
    ix = nc.dram_tensor("idx16", [P, NG * NSHARD * CC * P // 16], mybir.dt.int16,
                        kind="ExternalInput")
    rv = nc.dram_tensor("rowv", [P, T * KT], bf16, kind="ExternalInput")
    io = nc.dram_tensor("iota", [P, P], bf16, kind="ExternalInput")
    ot = nc.dram_tensor("out", [P, T * D], f32, kind="ExternalOutput")

    CLW = CC * P // 16

    with TileContext(nc) as tc:
        with (
            tc.tile_pool(name="const", bufs=1) as cpool,
            tc.tile_pool(name="gat", bufs=2) as gpool,
            tc.tile_pool(name="sel", bufs=3) as spool,
            tc.tile_pool(name="eplg", bufs=4) as epool,
            tc.tile_pool(name="acc", bufs=4, space="PSUM") as ppool,
        ):
            iota_sb = cpool.tile([P, P], bf16, tag="iota")
            nc.sync.dma_start(out=iota_sb[:, :], in_=io[:, :])
            idx_sb = cpool.tile([P, NG * NSHARD * CLW], mybir.dt.int16, tag="idxsb")
            nc.sync.dma_start(out=idx_sb[:, :], in_=ix[:, :])
            row_sb = cpool.tile([P, T * KT], bf16, tag="rowsb")
            nc.sync.dma_start(out=row_sb[:, :], in_=rv[:, :])
            slab_sb = cpool.tile([P, T * D], f32, tag="slab")
            nc.sync.dma_start(out=slab_sb[:, :], in_=fl[:, :])
            out_sb = cpool.tile([P, T * D], f32, tag="outsb")

            for g in range(NG):
                Gs = []
                for s in range(NSHARD):
                    Gt = gpool.tile([P, CC, W], bf16, tag=f"G{s}")
                    call = g * NSHARD + s
                    nc.gpsimd.dma_gather(
                        out_ap=Gt[:, :, :],
                        in_ap=tab[s * SS : min(N, (s + 1) * SS), :],
                        idxs_ap=idx_sb[:, call * CLW : (call + 1) * CLW],
                        num_idxs=CC * P,
                        num_idxs_reg=CC * P,
                        elem_size=W,
                        single_packet=False,
                    )
                    Gs.append(Gt)
                for tin in range(GRP):
                    t = g * GRP + tin
                    S = spool.tile([P, KT, P], bf16, tag="S")
                    nc.vector.tensor_tensor(
                        out=S[:, :, :],
                        in0=row_sb[:, t * KT : (t + 1) * KT]
                        .unsqueeze(-1)
                        .to_broadcast([P, KT, P]),
                        in1=iota_sb[:, :].unsqueeze(1).to_broadcast([P, KT, P]),
                        op=mybir.AluOpType.is_equal,
                    )
                    psum = ppool.tile([P, D + 1], f32, tag="psum")
                    for k in range(KT):
                        if k < NSHARD * cap:
                            s, c = k // cap, k % cap
                            rhs = Gs[s][:, tin * cap + c, 0 : D + 1]
                        else:
                            kk = k - NSHARD * cap
                            s, co = kk // GOC, kk % GOC
                            rhs = Gs[s][:, GRP * cap + co, 0 : D + 1]
                        nc.tensor.matmul(
                            out=psum[:, :],
                            lhsT=S[:, k, :],
                            rhs=rhs,
                            start=(k == 0),
                            stop=(k == KT - 1),
                        )
                    r = epool.tile([P, 1], f32, tag="recip")
                    nc.vector.tensor_scalar_max(
                        out=r[:, :], in0=psum[:, D : D + 1], scalar1=1.0
                    )
                    nc.vector.reciprocal(out=r[:, :], in_=r[:, :])
                    nc.vector.scalar_tensor_tensor(
                        out=out_sb[:, t * D : (t + 1) * D],
                        in0=psum[:, 0:D],
                        scalar=r[:, 0:1],
                        in1=slab_sb[:, t * D : (t + 1) * D],
                        op0=mybir.AluOpType.mult,
                        op1=mybir.AluOpType.add,
                    )
            nc.sync.dma_start(out=ot[:, :], in_=out_sb[:, :])
    nc.finalize()
    return nc


# ----------------------------------------------------------------- entry


def kernel(features, row, col):
    features = np.asarray(features, dtype=np.float32)
    n_cores = 8
    in_maps, meta = preprocess(features, row, col, n_cores)
    nc = build(meta)

    from concourse.bass_utils import run_bass_kernel_spmd

    res = run_bass_kernel_spmd(nc, in_maps, core_ids=list(range(n_cores)))
    return postprocess(res.results, meta)


if __name__ == "__main__":
    rng = np.random.default_rng(0)
    N, D, E = 7168, 64, 57344
    feats = rng.standard_normal((N, D), dtype=np.float32)
    row = rng.integers(0, N, E, dtype=np.int32)
    col = rng.integers(0, N, E, dtype=np.int32)
    out = kernel(feats, row, col)

    gathered = feats[col]
    summed = np.zeros((N, D), np.float32)
    np.add.at(summed, row, gathered)
    deg = np.clip(np.bincount(row, minlength=N).astype(np.float32), 1.0, None)
    exp = feats + summed / deg[:, None]
    rel = np.linalg.norm(out - exp) / np.linalg.norm(exp)
    print("rel err:", rel, "PASS" if rel < 5e-3 else "FAIL")
